# revision 16
# baseline (speedup 1.0000x reference)
"""AddTrend kernel for Trainium2 (8 NeuronCores, SPMD over batch).

out[b, s] = waveform[b, s] + c[b] * s
  where c[b] = max_abs[b] * slope[b] / (|slope[b]|*(S-1) + eps),
        slope[b] = tan(deg2rad(4*trend_deg[b] - 2)),
        max_abs[b] = max_s |waveform[b, s]|.

Only max_abs needs the device (a per-row abs-max reduction); the rest of the
per-row scalar math is done on host and shipped as `cpart[b] =
slope/(trend_max+eps)`. Each of the 8 cores owns 16 rows. Per row the core
loads the 2 MB row as a [128, 4096] tile, abs-max reduces on DVE, all-reduces
across partitions on GPSIMD, then fuses trend-multiply-add in one DVE
scalar_tensor_tensor: W = (X * c) + W, and stores. DMA-bound by design.
"""

import numpy as np

import concourse.bass as bass
import concourse.tile as tile
from concourse import bacc, bass_isa, mybir
from concourse.bass_utils import run_bass_kernel_spmd

N_CORES = 8
B, S = 128, 524288
RPC = B // N_CORES  # rows per core: 16
P = 128             # SBUF partitions
F = S // P          # free elems per partition: 4096
MIN_DEG, MAX_DEG, EPS = -2.0, 2.0, 1e-6

_cache: dict = {}


def _build(repeat: int = 1, variant: str = "full"):
    key = ("nc", repeat, variant)
    if key in _cache:
        return _cache[key]

    nc = bacc.Bacc(
        "TRN2", target_bir_lowering=False, debug=False, num_devices=N_CORES
    )
    f32 = mybir.dt.float32
    wave = nc.dram_tensor("wave", [RPC, S], f32, kind="ExternalInput").ap()
    cpart = nc.dram_tensor("cpart", [RPC], f32, kind="ExternalInput").ap()
    xgrid = nc.dram_tensor("xgrid", [S], f32, kind="ExternalInput").ap()
    out = nc.dram_tensor("out", [RPC, S], f32, kind="ExternalOutput").ap()

    wv = wave.rearrange("r (p f) -> r p f", p=P)
    ov = out.rearrange("r (p f) -> r p f", p=P)

    toks = variant.split(":")
    base = toks[0]
    flags = set(toks[1:])
    wbufs = 6
    for fl in flags:
        if fl.startswith("b"):
            wbufs = int(fl[1:])

    with tile.TileContext(nc) as tc:
        with (
            tc.tile_pool(name="const", bufs=1) as constp,
            tc.tile_pool(name="w", bufs=wbufs) as wp,
            tc.tile_pool(name="small", bufs=8) as sp,
        ):
            X = constp.tile([P, F], f32)
            nc.sync.dma_start(X[:], xgrid.rearrange("(p f) -> p f", p=P))

            cp_row = constp.tile([1, RPC], f32)
            nc.sync.dma_start(cp_row[:], cpart[None, :])
            cpB = constp.tile([P, RPC], f32)
            nc.gpsimd.partition_broadcast(cpB[:], cp_row[:], channels=P)

            store_eng = nc.sync
            load_eng = nc.sync
            if "sr" in flags:
                store_eng = nc.scalar
            if "sg" in flags:
                store_eng = nc.gpsimd
            if "lg" in flags:
                load_eng = nc.gpsimd
            if base == "storeonly":
                Wc = constp.tile([P, F], f32)
                nc.vector.memset(Wc[:], 1.0)

            if base.startswith("wide"):
                # Two rows per tile: [128, 2F] where cols [0,F) = row 2j and
                # [F,2F) = row 2j+1. Halves dma_start / POOL op counts.
                dp = int(base[4:]) if len(base) > 4 else 2
                NJ = RPC // 2
                wv3 = wave.rearrange(
                    "(j two) (p f) -> j p two f", two=2, p=P
                )
                ov3 = out.rearrange(
                    "(j two) (p f) -> j p two f", two=2, p=P
                )
                for rep in range(repeat):
                    Ws: dict[int, object] = {}
                    cs: dict[int, object] = {}
                    for j in range(NJ + dp):
                        if j < NJ:
                            W = wp.tile([P, 2, F], f32)
                            load_eng.dma_start(W[:], wv3[j])
                            m = sp.tile([P, 2], f32)
                            nc.vector.reduce_max(
                                m[:, 0:1], W[:, 0], mybir.AxisListType.X,
                                apply_absolute_value=True,
                            )
                            nc.vector.reduce_max(
                                m[:, 1:2], W[:, 1], mybir.AxisListType.X,
                                apply_absolute_value=True,
                            )
                            M = sp.tile([P, 2], f32)
                            nc.gpsimd.partition_all_reduce(
                                M[:], m[:], channels=P,
                                reduce_op=bass_isa.ReduceOp.max,
                            )
                            c = sp.tile([P, 2], f32)
                            nc.gpsimd.tensor_mul(
                                c[:], M[:], cpB[:, 2 * j : 2 * j + 2]
                            )
                            Ws[j], cs[j] = W, c
                        if j >= dp:
                            jb = j - dp
                            Wb, cb = Ws.pop(jb), cs.pop(jb)
                            for h in range(2):
                                nc.vector.scalar_tensor_tensor(
                                    Wb[:, h], X[:], cb[:, h : h + 1], Wb[:, h],
                                    op0=mybir.AluOpType.mult,
                                    op1=mybir.AluOpType.add,
                                )
                            store_eng.dma_start(ov3[jb], Wb[:])
                reps_left = 0
            elif base.startswith("half"):
                # Like pipe, but each row moves as two 1MB chunks for finer
                # load/store interleaving on the DMA fabric.
                d = int(base[4:]) if len(base) > 4 else 4
                H = F // 2
                for rep in range(repeat):
                    Ws: dict[int, object] = {}
                    cs: dict[int, object] = {}
                    for r in range(RPC + d):
                        if r < RPC:
                            W = wp.tile([P, F], f32)
                            load_eng.dma_start(
                                W[:, 0:H], wv[r][:, 0:H]
                            )
                            load_eng.dma_start(
                                W[:, H:F], wv[r][:, H:F]
                            )
                            mA = sp.tile([P, 1], f32)
                            nc.vector.reduce_max(
                                mA[:], W[:, 0:H], mybir.AxisListType.X,
                                apply_absolute_value=True,
                            )
                            mB = sp.tile([P, 1], f32)
                            nc.vector.reduce_max(
                                mB[:], W[:, H:F], mybir.AxisListType.X,
                                apply_absolute_value=True,
                            )
                            m = sp.tile([P, 1], f32)
                            nc.vector.tensor_max(m[:], mA[:], mB[:])
                            M = sp.tile([P, 1], f32)
                            nc.gpsimd.partition_all_reduce(
                                M[:], m[:], channels=P,
                                reduce_op=bass_isa.ReduceOp.max,
                            )
                            c = sp.tile([P, 1], f32)
                            nc.gpsimd.tensor_scalar_mul(
                                c[:], M[:], cpB[:, r : r + 1]
                            )
                            Ws[r], cs[r] = W, c
                        if r >= d:
                            rb = r - d
                            Wb, cb = Ws.pop(rb), cs.pop(rb)
                            nc.vector.scalar_tensor_tensor(
                                Wb[:, 0:H], X[:, 0:H], cb[:], Wb[:, 0:H],
                                op0=mybir.AluOpType.mult,
                                op1=mybir.AluOpType.add,
                            )
                            store_eng.dma_start(ov[rb][:, 0:H], Wb[:, 0:H])
                            nc.vector.scalar_tensor_tensor(
                                Wb[:, H:F], X[:, H:F], cb[:], Wb[:, H:F],
                                op0=mybir.AluOpType.mult,
                                op1=mybir.AluOpType.add,
                            )
                            store_eng.dma_start(ov[rb][:, H:F], Wb[:, H:F])
                reps_left = 0
            elif base.startswith("pipe"):
                # Software-pipelined: row r's scalar chain (abs-max reduce →
                # cross-partition max + scale on POOL) runs `d` rows ahead of
                # its trend-add + store, so DVE never stalls on POOL.
                d = int(base[4:]) if len(base) > 4 else 1
                for rep in range(repeat):
                    Ws: dict[int, object] = {}
                    cs: dict[int, object] = {}
                    for r in range(RPC + d):
                        if r < RPC:
                            W = wp.tile([P, F], f32)
                            load_eng.dma_start(W[:], wv[r])
                            m = sp.tile([P, 1], f32)
                            nc.vector.reduce_max(
                                m[:], W[:], mybir.AxisListType.X,
                                apply_absolute_value=True,
                            )
                            M = sp.tile([P, 1], f32)
                            nc.gpsimd.partition_all_reduce(
                                M[:], m[:], channels=P,
                                reduce_op=bass_isa.ReduceOp.max,
                            )
                            c = sp.tile([P, 1], f32)
                            nc.gpsimd.tensor_scalar_mul(
                                c[:], M[:], cpB[:, r : r + 1]
                            )
                            Ws[r], cs[r] = W, c
                        if r >= d:
                            rb = r - d
                            Wb, cb = Ws.pop(rb), cs.pop(rb)
                            nc.vector.scalar_tensor_tensor(
                                Wb[:], X[:], cb[:], Wb[:],
                                op0=mybir.AluOpType.mult,
                                op1=mybir.AluOpType.add,
                            )
                            store_eng.dma_start(ov[rb], Wb[:])
                reps_left = 0
            else:
                reps_left = repeat

            for rep in range(reps_left):
              for r in range(RPC):
                if base == "storeonly":
                    store_eng.dma_start(ov[r], Wc[:])
                    continue
                W = wp.tile([P, F], f32)
                load_eng.dma_start(W[:], wv[r])
                if base == "loadonly":
                    continue

                if base == "memcpy":
                    store_eng.dma_start(ov[r], W[:])
                    continue

                if base == "noreduce":
                    c = cpB[:, r : r + 1]
                else:
                    m = sp.tile([P, 1], f32)
                    nc.vector.reduce_max(
                        m[:], W[:], mybir.AxisListType.X,
                        apply_absolute_value=True,
                    )
                    if base == "nopool":
                        M = m
                    else:
                        M = sp.tile([P, 1], f32)
                        nc.gpsimd.partition_all_reduce(
                            M[:], m[:], channels=P,
                            reduce_op=bass_isa.ReduceOp.max,
                        )
                    c = sp.tile([P, 1], f32)
                    nc.vector.tensor_scalar_mul(c[:], M[:], cpB[:, r : r + 1])

                nc.vector.scalar_tensor_tensor(
                    W[:], X[:], c[:], W[:],
                    op0=mybir.AluOpType.mult, op1=mybir.AluOpType.add,
                )
                store_eng.dma_start(ov[r], W[:])

    nc.compile()
    _cache[key] = nc
    return nc


def _host_cpart(trend_deg: np.ndarray) -> np.ndarray:
    td = trend_deg.astype(np.float32)
    deg = np.float32(MAX_DEG - MIN_DEG) * td + np.float32(MIN_DEG)
    slope = np.tan(deg * np.float32(np.pi / 180.0)).astype(np.float32)
    trend_max = np.abs(slope * np.float32(S - 1))
    return (slope / (trend_max + np.float32(EPS))).astype(np.float32)


def kernel(waveform: np.ndarray, trend_deg: np.ndarray) -> np.ndarray:
    waveform = np.ascontiguousarray(waveform, dtype=np.float32)
    cpart = _host_cpart(np.asarray(trend_deg))
    xgrid = np.arange(S, dtype=np.float32)

    nc = _build(variant="pipe4")
    in_maps = [
        {
            "wave": waveform[i * RPC : (i + 1) * RPC],
            "cpart": np.ascontiguousarray(cpart[i * RPC : (i + 1) * RPC]),
            "xgrid": xgrid,
        }
        for i in range(N_CORES)
    ]
    res = run_bass_kernel_spmd(nc, in_maps, list(range(N_CORES)))
    return np.concatenate(
        [res.results[i]["out"] for i in range(N_CORES)], axis=0
    )
